# revision 1
# baseline (speedup 1.0000x reference)
"""Trainium2 Bass kernel for nn_Channel_CAM_38826504356088.

Math (validated against the reference to 2.5e-6 rel in fp32 numpy):
  rows = flattened (b, h, w); x viewed [rows, C] (NHWC natural layout)
  mean/var per channel over all rows (global over cores -> AllReduce)
  s = rsqrt(var + eps); bsig = -mean * s
  a = max(sigmoid(s*x + bsig), 0.5)        (== sigmoid(relu(batchnorm(x))))
  f = a @ w_down.T                          [rows, 16]
  G0 = f0.T @ f0 over batch-0 rows (global -> AllReduce)   [16, 16]
  out[oc, row] = sum_c (s_c*W1T[c,oc]) * x[c,row]      (x-term, s folded into W1)
               + bias_vec[oc]                           (-mean*s term, added at evac)
               + sum_j M2[j,oc] * f[j,row]              (Gram/channel-attention term)
  with W1 = w_final[:, :C], W2 = w_final[:, C:], M2 = ((W2 @ w_up) @ G0).T

Sharding: H split 8 ways (data-contiguous); per-core rows = 2*32*256 = 16384.
Per-core x.T is SBUF-resident as [C(partitions, 2 halves), rows] bf16; the
shards are cast to bf16 and transposed on the host (the device xbar-transpose
DMA measured ~2x slower than plain DMA on this runtime, and concurrent
transposes on the two HWDGE queues corrupt data). Output is produced in NCHW
layout directly from PSUM [oc, rows] tiles and upcast to f32 on the host.

Engine budget: GpSimd runs ONLY the collectives (any op queued behind a
collective stalls for its full latency, and the measured collective cost here
is ~125us each). Stats are split so they finish with the load: DVE bn_stats
(h0) + DVE sum-accumulate (h1) + ACT Square-accumulate (h1). Batch-1 phase-B
work is emitted after the G0 AllReduce so it overlaps the collective; the
BN bias folds into the PSUM-evacuation ops (per-partition bias add).
"""

import numpy as np

B = 2
H = 256
W = 256
C = 256
NCORES = 8
CH = 128          # channels per half (partition block)
RC = 512          # matmul row chunk (one PSUM bank, fp32)
OC2 = 1024        # output tile row-span (two PSUM banks)
BNC = 512         # bn_stats hardware chunk limit
BN_EPS = 1e-5


def build_kernel(rows, evac_dve_num=20, evac_dve_den=32, trace_sim=False):
    """Build the per-core SPMD Bass program. `rows` = B*H_shard*W per core."""
    from contextlib import ExitStack

    import concourse.bass as bass  # noqa: F401
    import concourse.tile as tile
    from concourse import bacc, mybir

    bf16 = mybir.dt.bfloat16
    f32 = mybir.dt.float32
    FT = mybir.ActivationFunctionType

    rows_b = rows // B            # rows per batch sample (batch-0 = first rows_b)
    rows_b0 = rows_b
    oc2 = min(OC2, rows_b)        # output tile row-span (<= two PSUM banks)
    n_oc2 = rows // oc2
    AC = min(2048, rows_b0)       # activation chunk; batch-0 chunks never straddle
    n_ac = rows // AC
    n_ac_b0 = rows_b0 // AC
    n_bn = rows // BNC
    dma_chunk = min(4096, rows)
    n_dc = rows // dma_chunk
    n_f0t = rows_b0 // 128        # number of 128-row f0T chunks

    nc = bacc.Bacc(
        "TRN2", target_bir_lowering=False, debug=False, num_devices=NCORES
    )

    xh = [
        nc.dram_tensor(f"xh{i}", [CH, rows], bf16, kind="ExternalInput").ap()
        for i in range(2)
    ]
    w1t_d = nc.dram_tensor("w1t", [C, C], f32, kind="ExternalInput").ap()
    wdt_d = nc.dram_tensor("wdt", [C, 16], bf16, kind="ExternalInput").ap()
    wu2t_d = nc.dram_tensor("wu2t", [16, C], bf16, kind="ExternalInput").ap()
    out_d = nc.dram_tensor("out", [B, C, rows_b], bf16, kind="ExternalOutput").ap()

    with tile.TileContext(nc, trace_sim=trace_sim) as tc, ExitStack() as ctx:
        ent = ctx.enter_context
        persist = ent(tc.tile_pool(name="persist", bufs=1))
        apool = ent(tc.tile_pool(name="acts", bufs=3))
        stats_pool = ent(tc.tile_pool(name="statsp", bufs=1))
        scrap = ent(tc.tile_pool(name="scrap", bufs=2))
        small = ent(tc.tile_pool(name="small", bufs=4))
        outp = ent(tc.tile_pool(name="outstage", bufs=4))
        ps_out = ent(tc.tile_pool(name="ps_out", bufs=2, space="PSUM"))
        ps_f = ent(tc.tile_pool(name="ps_f", bufs=2, space="PSUM"))
        ps_f0t = ent(tc.tile_pool(name="ps_f0t", bufs=1, space="PSUM"))
        ps_sm = ent(tc.tile_pool(name="ps_sm", bufs=1, space="PSUM"))
        dram = ent(tc.tile_pool(name="drambounce", bufs=1, space="DRAM"))

        # ---- persistent SBUF tensors
        xT = [
            persist.tile([CH, rows], bf16, name=f"xT{i}", tag=f"xT{i}")
            for i in range(2)
        ]
        f_s = persist.tile([16, rows], bf16, name="f_s", tag="f_s")
        f0t_s = persist.tile([CH, n_f0t * 16], bf16, name="f0t_s", tag="f0t_s")
        w1f = [
            persist.tile([CH, C], f32, name=f"w1f{i}", tag=f"w1f{i}")
            for i in range(2)
        ]
        w1s = [
            persist.tile([CH, C], bf16, name=f"w1s{i}", tag=f"w1s{i}")
            for i in range(2)
        ]
        wdt_s = [
            persist.tile([CH, 16], bf16, name=f"wdts{i}", tag=f"wdts{i}")
            for i in range(2)
        ]
        wu2t_s = persist.tile([16, C], bf16, name="wu2t_s", tag="wu2t_s")
        fw = persist.tile([16, C], bf16, name="fw", tag="fw")
        g0bf = persist.tile([16, 16], bf16, name="g0bf", tag="g0bf")
        g0gf = persist.tile([16, 16], f32, name="g0gf", tag="g0gf")
        eps_t = persist.tile([CH, 1], f32, name="eps_t", tag="eps_t")
        pay = persist.tile([CH, 4], f32, name="pay", tag="pay")
        pay_g = persist.tile([CH, 4], f32, name="pay_g", tag="pay_g")
        sv = [
            persist.tile([CH, 1], f32, name=f"sv{i}", tag=f"sv{i}") for i in range(2)
        ]
        bsig = [
            persist.tile([CH, 1], f32, name=f"bsig{i}", tag=f"bsig{i}")
            for i in range(2)
        ]
        nmean_bf = [
            persist.tile([CH, 1], bf16, name=f"nmean{i}", tag=f"nmean{i}")
            for i in range(2)
        ]
        bias_col = [
            persist.tile([CH, 1], f32, name=f"biascol{i}", tag=f"biascol{i}")
            for i in range(2)
        ]
        g0loc = persist.tile([16, 16], f32, name="g0loc", tag="g0loc")
        # stats partials: [half, dma-chunk]
        sum_p = persist.tile([CH, 2, n_dc], f32, name="sum_p", tag="sum_p")
        sq_p = persist.tile([CH, 2, n_dc], f32, name="sq_p", tag="sq_p")

        # ---- DRAM bounce buffers for collectives
        st_in = dram.tile([CH, 4], f32, name="st_in", tag="st_in")
        st_out = dram.tile([CH, 4], f32, name="st_out", tag="st_out")
        g0_in = dram.tile([16, 16], f32, name="g0_in", tag="g0_in")
        g0_out = dram.tile([16, 16], f32, name="g0_out", tag="g0_out")

        # ---- constants
        nc.vector.memset(eps_t, BN_EPS)

        # ---- weight loads
        for i in range(2):
            nc.sync.dma_start(out=w1f[i], in_=w1t_d[i * CH : (i + 1) * CH, :])
            nc.sync.dma_start(out=wdt_s[i], in_=wdt_d[i * CH : (i + 1) * CH, :])
        nc.sync.dma_start(out=wu2t_s, in_=wu2t_d[:, :])

        # ---- load x.T (host-side pre-transposed shards) with plain DMAs.
        # (Device-side xbar-transpose loads measured ~2x slower than plain
        # DMA on this runtime, and racing transposes across the two HWDGE
        # queues corrupts data — so the transpose moved to host sharding.)
        for j in range(n_dc):
            sl = slice(j * dma_chunk, (j + 1) * dma_chunk)
            nc.sync.dma_start(out=xT[0][:, sl], in_=xh[0][:, sl])
            nc.sync.dma_start(out=xT[1][:, sl], in_=xh[1][:, sl])

        # Stats, pipelined with the loads:
        #   h0 mean+var: DVE bn_stats
        #   h1 sum:      GpSimd tensor_scalar(+0) + accum_out (pre-collective)
        #   h1 sumsq:    ACT Square + accum_out
        bnst = stats_pool.tile([CH, n_bn, 6], f32, name="bnst0", tag="bnst0")
        for k in range(n_bn):
            nc.vector.bn_stats(
                out=bnst[:, k, :], in_=xT[0][:, k * BNC : (k + 1) * BNC]
            )
        for j in range(n_dc):
            sl = slice(j * dma_chunk, (j + 1) * dma_chunk)
            scr = scrap.tile(
                [CH, dma_chunk], bf16, name=f"scrs{j}", tag="scrs", bufs=1
            )
            nc.vector.tensor_scalar(
                out=scr,
                in0=xT[1][:, sl],
                scalar1=0.0,
                scalar2=None,
                op0=mybir.AluOpType.add,
                op1=mybir.AluOpType.add,
                accum_out=sum_p[:, 1, j : j + 1],
            )
            scr3 = scrap.tile(
                [CH, dma_chunk], bf16, name=f"scrq1_{j}", tag="scrq1", bufs=1
            )
            nc.scalar.activation(
                out=scr3,
                in_=xT[1][:, sl],
                func=FT.Square,
                accum_out=sq_p[:, 1, j : j + 1],
            )
        # payload: [mean, E[x^2]] per half, scaled 1/8 -> AllReduce(add) = global
        mv0 = small.tile([CH, 2], f32, name="mv0", tag="mv")
        nc.vector.bn_aggr(out=mv0, in_=bnst)
        tmp0 = small.tile([CH, 1], f32, name="tmsq0", tag="tmsq")
        nc.vector.tensor_scalar_mul(pay[:, 0:1], mv0[:, 0:1], 1.0 / NCORES)
        nc.vector.tensor_mul(tmp0, mv0[:, 0:1], mv0[:, 0:1])
        nc.vector.tensor_add(tmp0, tmp0, mv0[:, 1:2])
        nc.vector.tensor_scalar_mul(pay[:, 1:2], tmp0, 1.0 / NCORES)
        s1 = small.tile([CH, 1], f32, name="sum1", tag="tmsq")
        nc.vector.tensor_reduce(
            out=s1, in_=sum_p[:, 1, :], axis=mybir.AxisListType.X,
            op=mybir.AluOpType.add,
        )
        nc.vector.tensor_scalar_mul(pay[:, 2:3], s1, 1.0 / (NCORES * rows))
        q1 = small.tile([CH, 1], f32, name="sq1", tag="tmsq")
        nc.vector.tensor_reduce(
            out=q1, in_=sq_p[:, 1, :], axis=mybir.AxisListType.X,
            op=mybir.AluOpType.add,
        )
        nc.vector.tensor_scalar_mul(pay[:, 3:4], q1, 1.0 / (NCORES * rows))

        # ---- all-reduce the stats (GpSimd queue: collectives only)
        nc.sync.dma_start(out=st_in, in_=pay)
        nc.gpsimd.collective_compute(
            "AllReduce",
            mybir.AluOpType.add,
            replica_groups=[list(range(NCORES))],
            ins=[st_in.opt()],
            outs=[st_out.opt()],
        )
        nc.sync.dma_start(out=pay_g, in_=st_out)

        # ---- s, bsig, folded W1
        for i in range(2):
            mg = pay_g[:, 2 * i : 2 * i + 1]
            e2 = pay_g[:, 2 * i + 1 : 2 * i + 2]
            var = small.tile([CH, 1], f32, name=f"var{i}", tag="var")
            nc.vector.tensor_mul(var, mg, mg)
            nc.vector.tensor_sub(var, e2, var)
            sd = small.tile([CH, 1], f32, name=f"sd{i}", tag="sd")
            nc.scalar.activation(out=sd, in_=var, func=FT.Sqrt, bias=eps_t, scale=1.0)
            nc.vector.reciprocal(out=sv[i], in_=sd)
            nc.vector.tensor_scalar_mul(bsig[i], mg, -1.0)       # -mean
            nc.vector.tensor_copy(nmean_bf[i], bsig[i])          # bf16(-mean)
            nc.vector.tensor_mul(bsig[i], bsig[i], sv[i])        # -mean*s
            nc.vector.tensor_scalar_mul(w1s[i], w1f[i], sv[i])   # s*W1T (cast bf16)

        # bias_vec per oc-block: psum[oc,1] = sum_half (s*W1T).T @ (-mean)
        for oc in range(2):
            ocs = slice(oc * CH, (oc + 1) * CH)
            bp = ps_sm.tile([CH, 1], f32, name=f"biasps{oc}", tag="ps_small")
            nc.tensor.matmul(bp, w1s[0][:, ocs], nmean_bf[0], start=True, stop=False)
            nc.tensor.matmul(bp, w1s[1][:, ocs], nmean_bf[1], start=False, stop=True)
            nc.vector.tensor_copy(bias_col[oc], bp)

        # ---- phase B: activations, f, f0T
        def do_ac_chunk(ci):
            base = ci * AC
            a_t = []
            for i in range(2):
                at = apool.tile([CH, AC], bf16, name=f"a{i}_{ci}", tag=f"a{i}")
                nc.scalar.activation(
                    out=at,
                    in_=xT[i][:, base : base + AC],
                    func=FT.Sigmoid,
                    bias=bsig[i],
                    scale=sv[i],
                )
                nc.vector.tensor_scalar_max(at, at, 0.5)
                a_t.append(at)
            if base < rows_b0:  # f0T first: G0 is on the critical path
                for q in range(AC // 512):
                    pt = ps_f0t.tile([CH, 64], f32, name=f"psf0t_{ci}_{q}", tag="psf0t")
                    for j in range(4):
                        lsl = slice(q * 512 + j * 128, q * 512 + (j + 1) * 128)
                        psl = pt[:, j * 16 : (j + 1) * 16]
                        nc.tensor.matmul(
                            psl, a_t[0][:, lsl], wdt_s[0], start=True, stop=False
                        )
                        nc.tensor.matmul(
                            psl, a_t[1][:, lsl], wdt_s[1], start=False, stop=True
                        )
                    gq = ((base // 128) + q * 4) * 16
                    nc.vector.tensor_copy(f0t_s[:, gq : gq + 64], pt)
            for s_ in range(AC // RC):
                psf = ps_f.tile([16, RC], f32, name=f"psf_{ci}_{s_}", tag="psf")
                rsl = slice(s_ * RC, (s_ + 1) * RC)
                nc.tensor.matmul(psf, wdt_s[0], a_t[0][:, rsl], start=True, stop=False)
                nc.tensor.matmul(psf, wdt_s[1], a_t[1][:, rsl], start=False, stop=True)
                gsl = slice(base + s_ * RC, base + (s_ + 1) * RC)
                nc.vector.tensor_copy(f_s[:, gsl], psf)

        for ci in range(n_ac_b0):
            do_ac_chunk(ci)

        # ---- G0 (local) + all-reduce; batch-1 phase B overlaps the collective
        g0ps = ps_sm.tile([16, 16], f32, name="g0ps", tag="ps_small")
        for j in range(n_f0t):
            nc.tensor.matmul(
                g0ps,
                f0t_s[:, j * 16 : (j + 1) * 16],
                f0t_s[:, j * 16 : (j + 1) * 16],
                start=(j == 0),
                stop=(j == n_f0t - 1),
            )
        nc.vector.tensor_copy(g0loc, g0ps)
        nc.sync.dma_start(out=g0_in, in_=g0loc)
        nc.gpsimd.collective_compute(
            "AllReduce",
            mybir.AluOpType.add,
            replica_groups=[list(range(NCORES))],
            ins=[g0_in.opt()],
            outs=[g0_out.opt()],
        )
        nc.sync.dma_start(out=g0gf, in_=g0_out)
        nc.vector.tensor_copy(g0bf, g0gf)  # f32 -> bf16

        for ci in range(n_ac_b0, n_ac):
            do_ac_chunk(ci)

        # ---- M2 from the reduced G0: fw[j, oc] = sum_i G0[i,j] * Wu2T[i, oc]
        m2ps = ps_sm.tile([16, C], f32, name="m2ps", tag="ps_small")
        nc.tensor.matmul(m2ps, g0bf, wu2t_s, start=True, stop=True)
        nc.vector.tensor_copy(fw, m2ps)

        # ---- phase C: out tiles [oc, 2*RC] = x-term + Gram-term (+bias at evac)
        k = 0
        for oc in range(2):
            ocs = slice(oc * CH, (oc + 1) * CH)
            for t_i in range(n_oc2):
                r0 = t_i * oc2
                pso = ps_out.tile([CH, oc2], f32, name=f"pso_{oc}_{t_i}", tag="pso")
                for h_ in range(oc2 // RC):
                    rsl = slice(r0 + h_ * RC, r0 + (h_ + 1) * RC)
                    pss = pso[:, h_ * RC : (h_ + 1) * RC]
                    nc.tensor.matmul(
                        pss, w1s[0][:, ocs], xT[0][:, rsl], start=True, stop=False
                    )
                    nc.tensor.matmul(
                        pss, w1s[1][:, ocs], xT[1][:, rsl], start=False, stop=False
                    )
                    nc.tensor.matmul(
                        pss, fw[:, ocs], f_s[:, rsl], start=False, stop=True
                    )
                st = outp.tile([CH, oc2], bf16, name=f"ost_{oc}_{t_i}", tag="ost")
                if (k % evac_dve_den) < evac_dve_num:
                    nc.vector.tensor_scalar_add(st, pso, bias_col[oc])
                else:
                    nc.scalar.activation(
                        out=st, in_=pso, func=FT.Identity, bias=bias_col[oc], scale=1.0
                    )
                k += 1
                b_i = r0 // rows_b
                hw0 = r0 % rows_b
                nc.sync.dma_start(
                    out=out_d[b_i, oc * CH : (oc + 1) * CH, hw0 : hw0 + oc2], in_=st
                )

    nc.compile()
    return nc


_NC_CACHE = {}


def _get_nc(rows):
    if rows not in _NC_CACHE:
        _NC_CACHE[rows] = build_kernel(rows)
    return _NC_CACHE[rows]


def kernel(x, w_down, w_up, w_final):
    import ml_dtypes

    from concourse.bass_utils import run_bass_kernel_spmd

    bf16 = ml_dtypes.bfloat16
    x = np.asarray(x)
    w_down = np.asarray(w_down)
    w_up = np.asarray(w_up)
    w_final = np.asarray(w_final)

    # Host-side weight prep (tiny): fold W2 @ w_up; transpose for lhsT layouts.
    w1t = np.ascontiguousarray(w_final[:, :C].T).astype(np.float32)   # [256, 256]
    wdt = np.ascontiguousarray(w_down.T).astype(bf16)                 # [256, 16]
    wu2 = w_final[:, C:].astype(np.float32) @ w_up.astype(np.float32)  # [256, 16]
    wu2t = np.ascontiguousarray(wu2.T).astype(bf16)                   # [16, 256]

    HS = H // NCORES
    rows = B * HS * W
    in_maps = []
    for kcore in range(NCORES):
        xs = (
            np.ascontiguousarray(x[:, kcore * HS : (kcore + 1) * HS])
            .reshape(rows, C)
            .astype(bf16)
        )
        xt = np.ascontiguousarray(xs.T)  # [C, rows] per-core layout choice
        in_maps.append(
            {
                "xh0": np.ascontiguousarray(xt[:CH]),
                "xh1": np.ascontiguousarray(xt[CH:]),
                "w1t": w1t,
                "wdt": wdt,
                "wu2t": wu2t,
            }
        )

    nc = _get_nc(rows)
    res = run_bass_kernel_spmd(nc, in_maps, core_ids=list(range(NCORES)))

    out = np.empty((B, C, H, W), dtype=np.float32)
    rows_b = HS * W
    for kcore in range(NCORES):
        o = np.asarray(res.results[kcore]["out"]).astype(np.float32)
        out[:, :, kcore * HS : (kcore + 1) * HS, :] = o.reshape(B, C, HS, W)
    return out



# revision 3
# speedup vs baseline: 31198.9893x; 31198.9893x over previous
"""Trainium2 Bass kernel for nn_Channel_CAM_38826504356088.

Math (validated against the reference to 2.5e-6 rel in fp32 numpy):
  rows = flattened (b, h, w); x viewed [rows, C] (NHWC natural layout)
  mean/var per channel over all rows (global over cores -> AllReduce)
  s = rsqrt(var + eps); bsig = -mean * s
  a = max(sigmoid(s*x + bsig), 0.5)        (== sigmoid(relu(batchnorm(x))))
  f = a @ w_down.T                          [rows, 16]
  G0 = f0.T @ f0 over batch-0 rows (global -> AllReduce)   [16, 16]
  out[oc, row] = sum_c (s_c*W1T[c,oc]) * x[c,row]      (x-term, s folded into W1)
               + bias_vec[oc]                           (-mean*s term, added at evac)
               + sum_j M2[j,oc] * f[j,row]              (Gram/channel-attention term)
  with W1 = w_final[:, :C], W2 = w_final[:, C:], M2 = ((W2 @ w_up) @ G0).T

Sharding: H split 8 ways (data-contiguous); per-core rows = 2*32*256 = 16384.
Per-core x.T is SBUF-resident as [C(partitions, 2 halves), rows] bf16; the
shards are cast to bf16 and transposed on the host (the device xbar-transpose
DMA measured ~2x slower than plain DMA on this runtime, and concurrent
transposes on the two HWDGE queues corrupt data). Output is produced in NCHW
layout directly from PSUM [oc, rows] tiles and upcast to f32 on the host.

Engine budget: GpSimd runs ONLY the collectives (any op queued behind a
collective stalls for its full latency, and the measured collective cost here
is ~125us each). Stats are split so they finish with the load: DVE bn_stats
(h0) + DVE sum-accumulate (h1) + ACT Square-accumulate (h1). Batch-1 phase-B
work is emitted after the G0 AllReduce so it overlaps the collective; the
BN bias folds into the PSUM-evacuation ops (per-partition bias add).
"""

import numpy as np

B = 2
H = 256
W = 256
C = 256
NCORES = 8
CH = 128          # channels per half (partition block)
RC = 512          # matmul row chunk (one PSUM bank, fp32)
OC2 = 1024        # output tile row-span (two PSUM banks)
BNC = 512         # bn_stats hardware chunk limit
BN_EPS = 1e-5


def build_kernel(rows, evac_dve_num=20, evac_dve_den=32, trace_sim=False):
    """Build the per-core SPMD Bass program. `rows` = B*H_shard*W per core."""
    from contextlib import ExitStack

    import concourse.bass as bass  # noqa: F401
    import concourse.tile as tile
    from concourse import bacc, mybir

    bf16 = mybir.dt.bfloat16
    f32 = mybir.dt.float32
    FT = mybir.ActivationFunctionType

    rows_b = rows // B            # rows per batch sample (batch-0 = first rows_b)
    rows_b0 = rows_b
    oc2 = min(OC2, rows_b)        # output tile row-span (<= two PSUM banks)
    n_oc2 = rows // oc2
    AC = min(2048, rows_b0)       # activation chunk; batch-0 chunks never straddle
    n_ac = rows // AC
    n_ac_b0 = rows_b0 // AC
    n_bn = rows // BNC
    dma_chunk = min(4096, rows)
    n_dc = rows // dma_chunk
    n_f0t = rows_b0 // 128        # number of 128-row f0T chunks

    nc = bacc.Bacc(
        "TRN2", target_bir_lowering=False, debug=False, num_devices=NCORES
    )

    xh = [
        nc.dram_tensor(f"xh{i}", [CH, rows], bf16, kind="ExternalInput").ap()
        for i in range(2)
    ]
    w1t_d = nc.dram_tensor("w1t", [C, C], f32, kind="ExternalInput").ap()
    wdt_d = nc.dram_tensor("wdt", [C, 16], bf16, kind="ExternalInput").ap()
    wu2t_d = nc.dram_tensor("wu2t", [16, C], bf16, kind="ExternalInput").ap()
    out_d = nc.dram_tensor("out", [B, C, rows_b], bf16, kind="ExternalOutput").ap()

    with tile.TileContext(nc, trace_sim=trace_sim) as tc, ExitStack() as ctx:
        ent = ctx.enter_context
        persist = ent(tc.tile_pool(name="persist", bufs=1))
        apool = ent(tc.tile_pool(name="acts", bufs=3))
        stats_pool = ent(tc.tile_pool(name="statsp", bufs=1))
        scrap = ent(tc.tile_pool(name="scrap", bufs=2))
        small = ent(tc.tile_pool(name="small", bufs=4))
        outp = ent(tc.tile_pool(name="outstage", bufs=4))
        ps_out = ent(tc.tile_pool(name="ps_out", bufs=2, space="PSUM"))
        ps_f = ent(tc.tile_pool(name="ps_f", bufs=2, space="PSUM"))
        ps_f0t = ent(tc.tile_pool(name="ps_f0t", bufs=1, space="PSUM"))
        ps_sm = ent(tc.tile_pool(name="ps_sm", bufs=1, space="PSUM"))
        dram = ent(tc.tile_pool(name="drambounce", bufs=1, space="DRAM"))

        # ---- persistent SBUF tensors
        xT = [
            persist.tile([CH, rows], bf16, name=f"xT{i}", tag=f"xT{i}")
            for i in range(2)
        ]
        f_s = persist.tile([16, rows], bf16, name="f_s", tag="f_s")
        f0t_s = persist.tile([CH, n_f0t * 16], bf16, name="f0t_s", tag="f0t_s")
        w1f = [
            persist.tile([CH, C], f32, name=f"w1f{i}", tag=f"w1f{i}")
            for i in range(2)
        ]
        w1s = [
            persist.tile([CH, C], bf16, name=f"w1s{i}", tag=f"w1s{i}")
            for i in range(2)
        ]
        wdt_s = [
            persist.tile([CH, 16], bf16, name=f"wdts{i}", tag=f"wdts{i}")
            for i in range(2)
        ]
        wu2t_s = persist.tile([16, C], bf16, name="wu2t_s", tag="wu2t_s")
        fw = persist.tile([16, C], bf16, name="fw", tag="fw")
        g0bf = persist.tile([16, 16], bf16, name="g0bf", tag="g0bf")
        g0gf = persist.tile([16, 16], f32, name="g0gf", tag="g0gf")
        eps_t = persist.tile([CH, 1], f32, name="eps_t", tag="eps_t")
        pay = persist.tile([CH, 4], f32, name="pay", tag="pay")
        pay_g = persist.tile([CH, 4], f32, name="pay_g", tag="pay_g")
        sv = [
            persist.tile([CH, 1], f32, name=f"sv{i}", tag=f"sv{i}") for i in range(2)
        ]
        bsig = [
            persist.tile([CH, 1], f32, name=f"bsig{i}", tag=f"bsig{i}")
            for i in range(2)
        ]
        nmean_bf = [
            persist.tile([CH, 1], bf16, name=f"nmean{i}", tag=f"nmean{i}")
            for i in range(2)
        ]
        bias_col = [
            persist.tile([CH, 1], f32, name=f"biascol{i}", tag=f"biascol{i}")
            for i in range(2)
        ]
        g0loc = persist.tile([16, 16], f32, name="g0loc", tag="g0loc")
        # stats partials: [half, dma-chunk]
        sum_p = persist.tile([CH, 2, n_dc], f32, name="sum_p", tag="sum_p")
        sq_p = persist.tile([CH, 2, n_dc], f32, name="sq_p", tag="sq_p")

        # ---- DRAM bounce buffers for collectives
        st_in = dram.tile([CH, 4], f32, name="st_in", tag="st_in")
        st_out = dram.tile([CH, 4], f32, name="st_out", tag="st_out")
        g0_in = dram.tile([16, 16], f32, name="g0_in", tag="g0_in")
        g0_out = dram.tile([16, 16], f32, name="g0_out", tag="g0_out")

        # ---- constants
        nc.vector.memset(eps_t, BN_EPS)

        # ---- weight loads
        for i in range(2):
            nc.sync.dma_start(out=w1f[i], in_=w1t_d[i * CH : (i + 1) * CH, :])
            nc.sync.dma_start(out=wdt_s[i], in_=wdt_d[i * CH : (i + 1) * CH, :])
        nc.sync.dma_start(out=wu2t_s, in_=wu2t_d[:, :])

        # ---- load x.T (host-side pre-transposed shards) with plain DMAs.
        # (Device-side xbar-transpose loads measured ~2x slower than plain
        # DMA on this runtime, and racing transposes across the two HWDGE
        # queues corrupts data — so the transpose moved to host sharding.)
        for j in range(n_dc):
            sl = slice(j * dma_chunk, (j + 1) * dma_chunk)
            nc.sync.dma_start(out=xT[0][:, sl], in_=xh[0][:, sl])
            nc.sync.dma_start(out=xT[1][:, sl], in_=xh[1][:, sl])

        # Stats, pipelined with the loads:
        #   h0 mean+var: DVE bn_stats
        #   h1 sum:      GpSimd tensor_scalar(+0) + accum_out (pre-collective)
        #   h1 sumsq:    ACT Square + accum_out
        bnst = stats_pool.tile([CH, n_bn, 6], f32, name="bnst0", tag="bnst0")
        for k in range(n_bn):
            nc.vector.bn_stats(
                out=bnst[:, k, :], in_=xT[0][:, k * BNC : (k + 1) * BNC]
            )
        for j in range(n_dc):
            sl = slice(j * dma_chunk, (j + 1) * dma_chunk)
            scr = scrap.tile(
                [CH, dma_chunk], bf16, name=f"scrs{j}", tag="scrs", bufs=1
            )
            nc.vector.tensor_scalar(
                out=scr,
                in0=xT[1][:, sl],
                scalar1=0.0,
                scalar2=None,
                op0=mybir.AluOpType.add,
                op1=mybir.AluOpType.add,
                accum_out=sum_p[:, 1, j : j + 1],
            )
            scr3 = scrap.tile(
                [CH, dma_chunk], bf16, name=f"scrq1_{j}", tag="scrq1", bufs=1
            )
            nc.scalar.activation(
                out=scr3,
                in_=xT[1][:, sl],
                func=FT.Square,
                accum_out=sq_p[:, 1, j : j + 1],
            )
        # payload: [mean, E[x^2]] per half, scaled 1/8 -> AllReduce(add) = global
        mv0 = small.tile([CH, 2], f32, name="mv0", tag="mv")
        nc.vector.bn_aggr(out=mv0, in_=bnst)
        tmp0 = small.tile([CH, 1], f32, name="tmsq0", tag="tmsq")
        nc.vector.tensor_scalar_mul(pay[:, 0:1], mv0[:, 0:1], 1.0 / NCORES)
        nc.vector.tensor_mul(tmp0, mv0[:, 0:1], mv0[:, 0:1])
        nc.vector.tensor_add(tmp0, tmp0, mv0[:, 1:2])
        nc.vector.tensor_scalar_mul(pay[:, 1:2], tmp0, 1.0 / NCORES)
        s1 = small.tile([CH, 1], f32, name="sum1", tag="tmsq")
        nc.vector.tensor_reduce(
            out=s1, in_=sum_p[:, 1, :], axis=mybir.AxisListType.X,
            op=mybir.AluOpType.add,
        )
        nc.vector.tensor_scalar_mul(pay[:, 2:3], s1, 1.0 / (NCORES * rows))
        q1 = small.tile([CH, 1], f32, name="sq1", tag="tmsq")
        nc.vector.tensor_reduce(
            out=q1, in_=sq_p[:, 1, :], axis=mybir.AxisListType.X,
            op=mybir.AluOpType.add,
        )
        nc.vector.tensor_scalar_mul(pay[:, 3:4], q1, 1.0 / (NCORES * rows))

        # ---- all-reduce the stats (GpSimd queue: collectives only)
        nc.sync.dma_start(out=st_in, in_=pay)
        nc.gpsimd.collective_compute(
            "AllReduce",
            mybir.AluOpType.add,
            replica_groups=[list(range(NCORES))],
            ins=[st_in.opt()],
            outs=[st_out.opt()],
        )
        nc.sync.dma_start(out=pay_g, in_=st_out)

        # ---- s, bsig, folded W1
        for i in range(2):
            mg = pay_g[:, 2 * i : 2 * i + 1]
            e2 = pay_g[:, 2 * i + 1 : 2 * i + 2]
            var = small.tile([CH, 1], f32, name=f"var{i}", tag="var")
            nc.vector.tensor_mul(var, mg, mg)
            nc.vector.tensor_sub(var, e2, var)
            sd = small.tile([CH, 1], f32, name=f"sd{i}", tag="sd")
            nc.scalar.activation(out=sd, in_=var, func=FT.Sqrt, bias=eps_t, scale=1.0)
            nc.vector.reciprocal(out=sv[i], in_=sd)
            nc.vector.tensor_scalar_mul(bsig[i], mg, -1.0)       # -mean
            nc.vector.tensor_copy(nmean_bf[i], bsig[i])          # bf16(-mean)
            nc.vector.tensor_mul(bsig[i], bsig[i], sv[i])        # -mean*s
            nc.vector.tensor_scalar_mul(w1s[i], w1f[i], sv[i])   # s*W1T (cast bf16)

        # bias_vec per oc-block: psum[oc,1] = sum_half (s*W1T).T @ (-mean)
        for oc in range(2):
            ocs = slice(oc * CH, (oc + 1) * CH)
            bp = ps_sm.tile([CH, 1], f32, name=f"biasps{oc}", tag="ps_small")
            nc.tensor.matmul(bp, w1s[0][:, ocs], nmean_bf[0], start=True, stop=False)
            nc.tensor.matmul(bp, w1s[1][:, ocs], nmean_bf[1], start=False, stop=True)
            nc.vector.tensor_copy(bias_col[oc], bp)

        # ---- phase B: activations, f, f0T
        def do_ac_chunk(ci):
            base = ci * AC
            a_t = []
            for i in range(2):
                at = apool.tile([CH, AC], bf16, name=f"a{i}_{ci}", tag=f"a{i}")
                nc.scalar.activation(
                    out=at,
                    in_=xT[i][:, base : base + AC],
                    func=FT.Sigmoid,
                    bias=bsig[i],
                    scale=sv[i],
                )
                nc.vector.tensor_scalar_max(at, at, 0.5)
                a_t.append(at)
            if base < rows_b0:  # f0T first: G0 is on the critical path
                for q in range(AC // 512):
                    pt = ps_f0t.tile([CH, 64], f32, name=f"psf0t_{ci}_{q}", tag="psf0t")
                    for j in range(4):
                        lsl = slice(q * 512 + j * 128, q * 512 + (j + 1) * 128)
                        psl = pt[:, j * 16 : (j + 1) * 16]
                        nc.tensor.matmul(
                            psl, a_t[0][:, lsl], wdt_s[0], start=True, stop=False
                        )
                        nc.tensor.matmul(
                            psl, a_t[1][:, lsl], wdt_s[1], start=False, stop=True
                        )
                    gq = ((base // 128) + q * 4) * 16
                    nc.vector.tensor_copy(f0t_s[:, gq : gq + 64], pt)
            for s_ in range(AC // RC):
                psf = ps_f.tile([16, RC], f32, name=f"psf_{ci}_{s_}", tag="psf")
                rsl = slice(s_ * RC, (s_ + 1) * RC)
                nc.tensor.matmul(psf, wdt_s[0], a_t[0][:, rsl], start=True, stop=False)
                nc.tensor.matmul(psf, wdt_s[1], a_t[1][:, rsl], start=False, stop=True)
                gsl = slice(base + s_ * RC, base + (s_ + 1) * RC)
                nc.vector.tensor_copy(f_s[:, gsl], psf)

        for ci in range(n_ac_b0):
            do_ac_chunk(ci)

        # ---- G0 (local) + all-reduce; batch-1 phase B overlaps the collective
        g0ps = ps_sm.tile([16, 16], f32, name="g0ps", tag="ps_small")
        for j in range(n_f0t):
            nc.tensor.matmul(
                g0ps,
                f0t_s[:, j * 16 : (j + 1) * 16],
                f0t_s[:, j * 16 : (j + 1) * 16],
                start=(j == 0),
                stop=(j == n_f0t - 1),
            )
        nc.vector.tensor_copy(g0loc, g0ps)
        nc.sync.dma_start(out=g0_in, in_=g0loc)
        nc.gpsimd.collective_compute(
            "AllReduce",
            mybir.AluOpType.add,
            replica_groups=[list(range(NCORES))],
            ins=[g0_in.opt()],
            outs=[g0_out.opt()],
        )
        nc.sync.dma_start(out=g0gf, in_=g0_out)
        nc.vector.tensor_copy(g0bf, g0gf)  # f32 -> bf16

        for ci in range(n_ac_b0, n_ac):
            do_ac_chunk(ci)

        # ---- M2 from the reduced G0: fw[j, oc] = sum_i G0[i,j] * Wu2T[i, oc]
        m2ps = ps_sm.tile([16, C], f32, name="m2ps", tag="ps_small")
        nc.tensor.matmul(m2ps, g0bf, wu2t_s, start=True, stop=True)
        nc.vector.tensor_copy(fw, m2ps)

        # ---- phase C: out tiles [oc, 2*RC] = x-term + Gram-term (+bias at evac)
        k = 0
        for oc in range(2):
            ocs = slice(oc * CH, (oc + 1) * CH)
            for t_i in range(n_oc2):
                r0 = t_i * oc2
                pso = ps_out.tile([CH, oc2], f32, name=f"pso_{oc}_{t_i}", tag="pso")
                for h_ in range(oc2 // RC):
                    rsl = slice(r0 + h_ * RC, r0 + (h_ + 1) * RC)
                    pss = pso[:, h_ * RC : (h_ + 1) * RC]
                    nc.tensor.matmul(
                        pss, w1s[0][:, ocs], xT[0][:, rsl], start=True, stop=False
                    )
                    nc.tensor.matmul(
                        pss, w1s[1][:, ocs], xT[1][:, rsl], start=False, stop=False
                    )
                    nc.tensor.matmul(
                        pss, fw[:, ocs], f_s[:, rsl], start=False, stop=True
                    )
                st = outp.tile([CH, oc2], bf16, name=f"ost_{oc}_{t_i}", tag="ost")
                if (k % evac_dve_den) < evac_dve_num:
                    nc.vector.tensor_scalar_add(st, pso, bias_col[oc])
                else:
                    nc.scalar.activation(
                        out=st, in_=pso, func=FT.Identity, bias=bias_col[oc], scale=1.0
                    )
                k += 1
                b_i = r0 // rows_b
                hw0 = r0 % rows_b
                nc.sync.dma_start(
                    out=out_d[b_i, oc * CH : (oc + 1) * CH, hw0 : hw0 + oc2], in_=st
                )

    nc.compile()
    return nc


_NC_CACHE = {}


def _get_nc(rows):
    if rows not in _NC_CACHE:
        _NC_CACHE[rows] = build_kernel(rows)
    return _NC_CACHE[rows]


def prepare(x, w_down, w_up, w_final):
    """Host-side shard prep; returns (in_maps, nc, unshard_fn)."""
    import ml_dtypes

    bf16 = ml_dtypes.bfloat16
    x = np.asarray(x)
    w_down = np.asarray(w_down)
    w_up = np.asarray(w_up)
    w_final = np.asarray(w_final)

    # Host-side weight prep (tiny): fold W2 @ w_up; transpose for lhsT layouts.
    w1t = np.ascontiguousarray(w_final[:, :C].T).astype(np.float32)   # [256, 256]
    wdt = np.ascontiguousarray(w_down.T).astype(bf16)                 # [256, 16]
    wu2 = w_final[:, C:].astype(np.float32) @ w_up.astype(np.float32)  # [256, 16]
    wu2t = np.ascontiguousarray(wu2.T).astype(bf16)                   # [16, 256]

    HS = H // NCORES
    rows = B * HS * W
    in_maps = []
    for kcore in range(NCORES):
        xs = (
            np.ascontiguousarray(x[:, kcore * HS : (kcore + 1) * HS])
            .reshape(rows, C)
            .astype(bf16)
        )
        xt = np.ascontiguousarray(xs.T)  # [C, rows] per-core layout choice
        in_maps.append(
            {
                "xh0": np.ascontiguousarray(xt[:CH]),
                "xh1": np.ascontiguousarray(xt[CH:]),
                "w1t": w1t,
                "wdt": wdt,
                "wu2t": wu2t,
            }
        )

    nc = _get_nc(rows)

    def unshard(results):
        out = np.empty((B, C, H, W), dtype=np.float32)
        rows_b = HS * W
        for kcore in range(NCORES):
            o = np.asarray(results[kcore]["out"]).astype(np.float32)
            out[:, :, kcore * HS : (kcore + 1) * HS, :] = o.reshape(B, C, HS, W)
        return out

    return in_maps, nc, unshard


def kernel(x, w_down, w_up, w_final):
    from concourse.bass_utils import run_bass_kernel_spmd

    in_maps, nc, unshard = prepare(x, w_down, w_up, w_final)
    res = run_bass_kernel_spmd(nc, in_maps, core_ids=list(range(NCORES)))
    return unshard(res.results)

